# revision 24
# baseline (speedup 1.0000x reference)
"""Trainium2 Bass kernel for a 4-layer GRU stack with per-step additive
self-attention over the layer hiddens (FBRNN).

Strategy: data-parallel over batch B=64 across 8 NeuronCores (8 batch rows
per core, no cross-core communication inside the recurrence). Per core:

  - Everything lives in a [feature-on-partitions, batch-on-free] layout so
    the GRU elementwise runs on 128 DVE/ACT lanes.
  - GRU matmuls: stationary operand = bf16 weight tiles [128,128] (FWL),
    moving operand = bf16 activations [128, 8]. PSUM accumulates fp32.
  - Layer-0 input transform (x @ W_ih[0].T) has no recurrent dependency:
    it is precomputed for all T in a batched GEMM at kernel start (after an
    on-device embedding gather via indirect DMA + PE transposes), stored in
    DRAM, and streamed back 49KB/step.
  - sigmoid(x) = 0.5*tanh(0.5x)+0.5 so the whole kernel uses one ACT
    table set (exp_and_others: tanh+exp) -> no ~2.7us table switches.
  - T-loop: tc.For_i with 8 steps unrolled per iteration.
"""

import os
import numpy as np
import ml_dtypes

import concourse.bass as bass
import concourse.mybir as mybir
import concourse.tile as tile
from concourse import bacc
from concourse.bass import ds, ts
from concourse.bass_utils import run_bass_kernel_spmd
from concourse.masks import make_identity

F32 = mybir.dt.float32
BF16 = mybir.dt.bfloat16
I32 = mybir.dt.int32
AF = mybir.ActivationFunctionType
ALU = mybir.AluOpType

T, B = 512, 64
V, E, H, L, A = 32000, 512, 512, 4, 256
NCORES = 8
BC = B // NCORES            # 8 batch rows per core
TOK = T * BC                # 4096 tokens per core, (t, b) order
G3 = 3 * H                  # 1536 gate rows
MCH = G3 // 128             # 12 gate chunks
KCH = E // 128              # 4 contraction chunks (E == H)
ACH = A // 128              # 2 attention chunks
HT = H // 128               # 4 hidden chunks
UNROLL = 8
SLAB = 512                  # tokens per prologue gemm slab
DEBUG_H = False             # add per-step dump of the full h state

# attention pair-block offsets for i=0..2 (i=3 is identity); block i holds
# columns (b, k) for k in [i, 4), b-major; block size (4-i)*BC
_OFF = [0, 4 * BC, 7 * BC]
_ETOT = 9 * BC              # 72


def _bcast(ap, dim, count):
    """Insert a [step=0, count] free dim at position `dim` (0=partition)."""
    new = list(ap.ap)
    new.insert(dim, [0, count])
    return bass.AP(tensor=ap.tensor, offset=ap.offset, ap=new)


def _build_kernel():
    nc = bacc.Bacc("TRN2", target_bir_lowering=False, debug=False)

    tokens_d = nc.dram_tensor("tokens32", [TOK // 128, 128], I32, kind="ExternalInput").ap()
    emb_d = nc.dram_tensor("embbf", [V, E], BF16, kind="ExternalInput").ap()
    wih0_d = nc.dram_tensor("wih0", [128, KCH, MCH, 128], BF16, kind="ExternalInput").ap()
    wih_d = nc.dram_tensor("wih", [L - 1, 128, KCH, MCH, 128], BF16, kind="ExternalInput").ap()
    whh_d = nc.dram_tensor("whh", [L, 128, KCH, MCH, 128], BF16, kind="ExternalInput").ap()
    wa_d = nc.dram_tensor("wa", [L, 128, KCH, ACH, 128], BF16, kind="ExternalInput").ap()
    va_d = nc.dram_tensor("vastk", [128, ACH, L], BF16, kind="ExternalInput").ap()
    ba_d = nc.dram_tensor("bastk", [128, ACH, L], F32, kind="ExternalInput").ap()
    brz_d = nc.dram_tensor("brz", [L, 128, 8], F32, kind="ExternalInput").ap()
    bin_d = nc.dram_tensor("bin", [L, 128, HT], F32, kind="ExternalInput").ap()
    bhn_d = nc.dram_tensor("bhn", [L, 128, HT], F32, kind="ExternalInput").ap()
    out_d = nc.dram_tensor("out", [T * BC, H], F32, kind="ExternalOutput").ap()
    global _dbg_d
    _dbg_d = None
    if DEBUG_H:
        _dbg_d = nc.dram_tensor("dbg", [T, 2, 128, HT * BC * L], F32,
                                kind="ExternalOutput").ap()

    with tile.TileContext(nc) as tc:
        _emit(tc, nc, tokens_d, emb_d, wih0_d, wih_d, whh_d, wa_d, va_d, ba_d,
              brz_d, bin_d, bhn_d, out_d)
    nc.compile()
    return nc


def _emit(tc, nc, tokens_d, emb_d, wih0_d, wih_d, whh_d, wa_d, va_d, ba_d,
          brz_d, bin_d, bhn_d, out_d):
    from contextlib import ExitStack

    ctx = ExitStack()
    with ctx:
        wpool = ctx.enter_context(tc.tile_pool(name="weights", bufs=1))
        state = ctx.enter_context(tc.tile_pool(name="state", bufs=1))
        dram = ctx.enter_context(tc.tile_pool(name="dram", bufs=1, space="DRAM"))

        # ---- resident weights -------------------------------------------
        wih0_sb = wpool.tile([128, KCH, MCH, 128], BF16, tag="wih0")
        nc.sync.dma_start(out=wih0_sb, in_=wih0_d)
        wih_sb = []
        for l in range(L - 1):
            w = wpool.tile([128, KCH, MCH, 128], BF16, tag=f"wih{l}")
            nc.sync.dma_start(out=w, in_=wih_d[l])
            wih_sb.append(w)
        whh_sb = []
        for l in range(L):
            w = wpool.tile([128, KCH, MCH, 128], BF16, tag=f"whh{l}")
            nc.sync.dma_start(out=w, in_=whh_d[l])
            whh_sb.append(w)
        wa_sb = []
        for i in range(L):
            w = wpool.tile([128, KCH, ACH, 128], BF16, tag=f"wa{i}")
            nc.sync.dma_start(out=w, in_=wa_d[i])
            wa_sb.append(w)
        va_sb = wpool.tile([128, ACH, L], BF16, tag="va")
        nc.sync.dma_start(out=va_sb, in_=va_d)
        ba_sb = wpool.tile([128, ACH, L], F32, tag="ba")
        nc.sync.dma_start(out=ba_sb, in_=ba_d)
        brz_sb = wpool.tile([128, L, 8], F32, tag="brz")
        nc.sync.dma_start(out=brz_sb, in_=brz_d.rearrange("l p m -> p l m"))
        bin_sb = wpool.tile([128, L, HT], F32, tag="bin")
        nc.sync.dma_start(out=bin_sb, in_=bin_d.rearrange("l p m -> p l m"))
        bhn_sb = wpool.tile([128, L, HT], F32, tag="bhn")
        nc.sync.dma_start(out=bhn_sb, in_=bhn_d.rearrange("l p m -> p l m"))

        ident = wpool.tile([128, 128], BF16, tag="ident")
        make_identity(nc, ident)
        ones_sb = wpool.tile([1, 128], BF16, tag="ones")
        nc.vector.memset(ones_sb, 1.0)

        # ---- recurrent state --------------------------------------------
        # layout: [128 part, HT, BC, L]
        h_f32 = state.tile([128, HT, BC, L], F32, tag="h_f32")
        h_bf = state.tile([128, HT, BC, L], BF16, tag="h_bf")
        new_f32 = state.tile([128, HT, BC, L], F32, tag="new_f32")
        new_bf = state.tile([128, HT, BC, L], BF16, tag="new_bf")
        nc.vector.memset(h_f32, 0.0)
        nc.vector.memset(h_bf, 0.0)

        # gi0[m, p, tok] fp32: precomputed x @ W_ih[0].T (no bias)
        gi0_dram = dram.tile([MCH, 128, TOK], F32, tag="gi0")

        # ---- prologue: embedding gather + layer-0 input GEMM ------------
        with tc.tile_pool(name="prol", bufs=2) as prol, \
             tc.tile_pool(name="prol_ps", bufs=2, space="PSUM") as prol_ps, \
             tc.tile_pool(name="gemm_ps", bufs=2, space="PSUM") as gemm_ps, \
             tc.tile_pool(name="evac", bufs=2) as evac, \
             tc.tile_pool(name="x0t", bufs=2) as x0tp:
            for slab in range(TOK // SLAB):
                x0t = x0tp.tile([128, KCH, SLAB], BF16, tag="x0t")
                for g in range(SLAB // 128):
                    gt = slab * (SLAB // 128) + g
                    tok_sb = prol.tile([128, 1], I32, tag="tok")
                    nc.sync.dma_start(out=tok_sb, in_=tokens_d[gt, :, None])
                    x0 = prol.tile([128, E], BF16, tag="x0")
                    nc.gpsimd.indirect_dma_start(
                        out=x0, out_offset=None, in_=emb_d,
                        in_offset=bass.IndirectOffsetOnAxis(ap=tok_sb[:, 0:1], axis=0),
                    )
                    for k in range(KCH):
                        pst = prol_ps.tile([128, 128], BF16, space="PSUM", tag="pst")
                        nc.tensor.transpose(out=pst, in_=x0[:, ts(k, 128)], identity=ident)
                        nc.vector.tensor_copy(out=x0t[:, k, ts(g, 128)], in_=pst)
                for m in range(MCH):
                    ps = gemm_ps.tile([128, SLAB], F32, space="PSUM", tag="g0ps")
                    for k in range(KCH):
                        nc.tensor.matmul(
                            out=ps, lhsT=wih0_sb[:, k, m, :], rhs=x0t[:, k, :],
                            start=(k == 0), stop=(k == KCH - 1),
                        )
                    ev = evac.tile([128, SLAB], F32, tag="ev")
                    nc.scalar.activation(out=ev, in_=ps, func=AF.Copy)
                    nc.sync.dma_start(out=gi0_dram[m, :, ts(slab, SLAB)], in_=ev)

        # ---- main recurrence --------------------------------------------
        loop_pools = ExitStack()
        with loop_pools:
            gip = loop_pools.enter_context(tc.tile_pool(name="gi", bufs=3))
            pgp = loop_pools.enter_context(tc.tile_pool(name="pg", bufs=4, space="PSUM"))
            ep = loop_pools.enter_context(tc.tile_pool(name="elem", bufs=3))
            up = loop_pools.enter_context(tc.tile_pool(name="ups", bufs=1, space="PSUM"))
            ep2 = loop_pools.enter_context(tc.tile_pool(name="eps", bufs=1, space="PSUM"))
            ep3 = loop_pools.enter_context(tc.tile_pool(name="abcps", bufs=1, space="PSUM"))
            ap_ = loop_pools.enter_context(tc.tile_pool(name="attn", bufs=2))
            pp = loop_pools.enter_context(tc.tile_pool(name="prod", bufs=2))

            with tc.For_i(0, TOK, BC * UNROLL,
                          hint_engines=(mybir.EngineType.PE,
                                        mybir.EngineType.DVE,
                                        mybir.EngineType.Activation)) as iv:
                for u in range(UNROLL):
                    _step(tc, nc, iv, u, gip, pgp, ep, up, ep2, ep3, ap_, pp,
                          wih_sb, whh_sb, wa_sb, va_sb, ba_sb, brz_sb, bin_sb,
                          bhn_sb, ones_sb, h_f32, h_bf, new_f32, new_bf,
                          gi0_dram, out_d)


def _step(tc, nc, iv, u, gip, pgp, ep, up, ep2, ep3, ap_, pp,
          wih_sb, whh_sb, wa_sb, va_sb, ba_sb, brz_sb, bin_sb, bhn_sb,
          ones_sb, h_f32, h_bf, new_f32, new_bf, gi0_dram, out_d):
    tb0 = iv + u * BC  # token index of (t, b=0)

    # stream in the precomputed layer-0 gi for this step: [128, MCH, BC]
    gi_sb = gip.tile([128, MCH, BC], F32, tag="gi0s")
    nc.sync.dma_start(
        out=gi_sb,
        in_=gi0_dram[:, :, ds(tb0, BC)].rearrange("m p b -> p m b"),
    )

    # psum gate tiles per layer: slots 0..7 = gh r,z; 8..11 = gh n-part;
    # slots 12..19 = gi r,z; 20..23 = gi n-part.  [128, 24, BC] = 1 bank.
    pg = [pgp.tile([128, 24, BC], F32, space="PSUM", tag="pg", name=f"pg{_l}")
          for _l in range(L)]

    def mm_gh(l):
        for m in range(MCH):
            for k in range(KCH):
                nc.tensor.matmul(
                    out=pg[l][:, m, :],
                    lhsT=whh_sb[l][:, k, m, :],
                    rhs=h_bf[:, k, :, l],
                    start=(k == 0) and m == 0,
                    stop=(k == KCH - 1) and m == MCH - 1,
                    skip_group_check=True,
                )

    def mm_gi(l):  # l >= 1; input = new[l-1]
        for m in range(MCH):
            for k in range(KCH):
                nc.tensor.matmul(
                    out=pg[l][:, 12 + m, :],
                    lhsT=wih_sb[l - 1][:, k, m, :],
                    rhs=new_bf[:, k, :, l - 1],
                    start=(k == 0) and m == 0,
                    stop=(k == KCH - 1) and m == MCH - 1,
                    skip_group_check=True,
                )

    def elem(l):
        # rz = sigmoid(gi_rz + gh_rz + b_rz) via 0.5*tanh(0.5x)+0.5
        # (walrus: each TensorTensor may read at most one PSUM operand)
        girz = gi_sb[:, 0:8, :] if l == 0 else pg[l][:, 12:20, :]
        rzb = ep.tile([128, 8, BC], F32, tag="rzb")
        nc.vector.tensor_tensor(out=rzb, in0=pg[l][:, 0:8, :],
                                in1=_bcast(brz_sb[:, l, :], 2, BC), op=ALU.add)
        nc.vector.tensor_tensor(out=rzb, in0=rzb, in1=girz, op=ALU.add)
        trz = ep.tile([128, 8, BC], F32, tag="trz")
        nc.scalar.activation(out=trz, in_=rzb, func=AF.Tanh, scale=0.5)
        rz = ep.tile([128, 8, BC], F32, tag="rz")
        nc.vector.tensor_scalar(out=rz, in0=trz, scalar1=0.5, scalar2=0.5,
                                op0=ALU.mult, op1=ALU.add)
        # n = tanh(gi_n + b_in + r * (gh_n + b_hn))
        hnb = ep.tile([128, HT, BC], F32, tag="hnb")
        nc.vector.tensor_tensor(out=hnb, in0=pg[l][:, 8:12, :],
                                in1=_bcast(bhn_sb[:, l, :], 2, BC), op=ALU.add)
        rh = ep.tile([128, HT, BC], F32, tag="rh")
        nc.vector.tensor_tensor(out=rh, in0=rz[:, 0:4, :], in1=hnb, op=ALU.mult)
        np1 = ep.tile([128, HT, BC], F32, tag="np1")
        gin = gi_sb[:, 8:12, :] if l == 0 else pg[l][:, 20:24, :]
        nc.vector.tensor_tensor(out=np1, in0=rh, in1=gin, op=ALU.add)
        np2 = ep.tile([128, HT, BC], F32, tag="np2")
        nc.vector.tensor_tensor(out=np2, in0=np1,
                                in1=_bcast(bin_sb[:, l, :], 2, BC), op=ALU.add)
        n = ep.tile([128, HT, BC], F32, tag="n")
        nc.scalar.activation(out=n, in_=np2, func=AF.Tanh)
        # new = n + z*(h - n)
        d = ep.tile([128, HT, BC], F32, tag="d")
        nc.vector.tensor_tensor(out=d, in0=h_f32[:, :, :, l], in1=n, op=ALU.subtract)
        zd = ep.tile([128, HT, BC], F32, tag="zd")
        nc.vector.tensor_tensor(out=zd, in0=rz[:, 4:8, :], in1=d, op=ALU.mult)
        nc.vector.tensor_tensor(out=new_f32[:, :, :, l], in0=n, in1=zd, op=ALU.add)
        nc.vector.tensor_copy(out=new_bf[:, :, :, l], in_=new_f32[:, :, :, l])

    # PE order: gh0, gh1, gi1, gh2, gi2, gh3, gi3 (gi[l] gated on elem[l-1])
    mm_gh(0)
    mm_gh(1)
    elem(0)
    mm_gi(1)
    mm_gh(2)
    elem(1)
    mm_gi(2)
    mm_gh(3)
    elem(2)
    mm_gi(3)
    elem(3)

    # ---- attention combine ------------------------------------------
    # u[i,k] = Wa[i].T @ new[k]  for i<3, k>=i; columns (b, k) per block i
    u_ps = up.tile([128, ACH, _ETOT], F32, space="PSUM", tag="ups")
    for i in range(3):
        sz = (L - i) * BC
        for a2 in range(ACH):
            for k in range(KCH):
                nc.tensor.matmul(
                    out=u_ps[:, a2, _OFF[i]:_OFF[i] + sz],
                    lhsT=wa_sb[i][:, k, a2, :],
                    rhs=new_bf[:, k, :, i:L],
                    start=(k == 0), stop=(k == KCH - 1),
                    skip_group_check=True,
                )
    u_sb = ap_.tile([128, ACH, _ETOT], F32, tag="usb")
    for i in range(3):
        sz = (L - i) * BC
        nc.vector.tensor_tensor(
            out=u_sb[:, :, _OFF[i]:_OFF[i] + sz],
            in0=u_ps[:, :, _OFF[i]:_OFF[i] + sz],
            in1=_bcast(ba_sb[:, :, i], 2, sz),
            op=ALU.add,
        )
    ut = ap_.tile([128, ACH, _ETOT], BF16, tag="ut")
    nc.scalar.activation(out=ut, in_=u_sb, func=AF.Tanh)
    e_ps = ep2.tile([1, _ETOT], F32, space="PSUM", tag="eps")
    for i in range(3):
        sz = (L - i) * BC
        for a2 in range(ACH):
            nc.tensor.matmul(out=e_ps[0:1, _OFF[i]:_OFF[i] + sz],
                             lhsT=va_sb[:, a2, i:i + 1],
                             rhs=ut[:, a2, _OFF[i]:_OFF[i] + sz],
                             start=(a2 == 0), stop=(a2 == ACH - 1),
                             skip_group_check=True)
    ee = ap_.tile([1, _ETOT], F32, tag="ee")
    nc.scalar.activation(out=ee, in_=e_ps, func=AF.Exp)
    s_all = ap_.tile([1, 4, BC], F32, tag="sall")
    for i in range(3):
        kk = L - i
        nc.vector.tensor_reduce(
            out=s_all[0:1, i, :],
            in_=ee[0:1, _OFF[i]:_OFF[i] + kk * BC].rearrange(
                "p (b k) -> p b k", k=kk),
            axis=mybir.AxisListType.X, op=ALU.add,
        )
    rs = ap_.tile([1, 4, BC], F32, tag="rs")
    nc.vector.reciprocal(out=rs[0:1, 0:3, :], in_=s_all[0:1, 0:3, :])
    a_bf = ap_.tile([1, _ETOT], BF16, tag="abf")
    for i in range(3):
        kk = L - i
        nc.vector.tensor_tensor(
            out=a_bf[0:1, _OFF[i]:_OFF[i] + kk * BC].rearrange(
                "p (b k) -> p b k", k=kk),
            in0=ee[0:1, _OFF[i]:_OFF[i] + kk * BC].rearrange(
                "p (b k) -> p b k", k=kk),
            in1=_bcast(rs[0:1, i, :], 2, kk),
            op=ALU.mult,
        )
    abc_ps = ep3.tile([128, _ETOT], F32, space="PSUM", tag="abc")
    nc.tensor.matmul(out=abc_ps, lhsT=ones_sb, rhs=a_bf, start=True, stop=True)
    for i in range(3):
        kk = L - i
        prod = pp.tile([128, HT, BC, L], F32, tag="prod")
        av = abc_ps[:, _OFF[i]:_OFF[i] + kk * BC].rearrange("p (b k) -> p b k", k=kk)
        nc.vector.tensor_tensor(
            out=prod[:, :, :, 0:kk],
            in0=new_f32[:, :, :, i:L],
            in1=_bcast(av, 1, HT),
            op=ALU.mult,
        )
        nc.vector.tensor_reduce(out=h_f32[:, :, :, i], in_=prod[:, :, :, 0:kk],
                                axis=mybir.AxisListType.X, op=ALU.add)
        nc.vector.tensor_copy(out=h_bf[:, :, :, i], in_=h_f32[:, :, :, i])
    # i = 3: softmax over a single element -> h_next[3] = new[3]
    nc.vector.tensor_copy(out=h_f32[:, :, :, 3], in_=new_f32[:, :, :, 3])
    nc.vector.tensor_copy(out=h_bf[:, :, :, 3], in_=new_f32[:, :, :, 3])

    # output row block: out[(t,b), :] for this step's 8 batch rows
    if DEBUG_H:
        # row offset t*256 = iv*32 + u*256 (iv counts tokens, 8/step)
        nc.sync.dma_start(
            out=_dbg_d.rearrange("t s p f -> (t s p) f")[
                ds(iv * 32 + u * 256, 128), :],
            in_=h_f32.rearrange("p ht b l -> p (ht b l)"),
        )
        nc.sync.dma_start(
            out=_dbg_d.rearrange("t s p f -> (t s p) f")[
                ds(iv * 32 + u * 256 + 128, 128), :],
            in_=new_f32.rearrange("p ht b l -> p (ht b l)"),
        )
    out_stage = ap_.tile([128, BC, HT], F32, tag="ostage")
    nc.vector.tensor_copy(out=out_stage,
                          in_=h_f32[:, :, :, 3].rearrange("p ht b -> p b ht"))
    nc.sync.dma_start(
        out=out_d[ds(tb0, BC), :].rearrange("b (ht p) -> p b ht", p=128),
        in_=out_stage,
    )


_NC_CACHE = {}


def _get_nc():
    if "nc" not in _NC_CACHE:
        _NC_CACHE["nc"] = _build_kernel()
    return _NC_CACHE["nc"]


def _prep_inputs(tokens, emb, W_ih, W_hh, b_ih, b_hh, Wa, ba, va):
    """Host-side input marshalling (weight layout/dtype only, no compute)."""
    bf = ml_dtypes.bfloat16
    emb_bf = np.ascontiguousarray(np.asarray(emb, np.float32).astype(bf))

    def lhsT_layout(wT):  # [K, M] -> [128, KCH, MCH, 128]
        K, M = wT.shape
        return np.ascontiguousarray(
            wT.reshape(K // 128, 128, M // 128, 128).transpose(1, 0, 2, 3).astype(bf))

    wih_t = [lhsT_layout(np.asarray(W_ih[l], np.float32).T) for l in range(L)]
    whh_t = [lhsT_layout(np.asarray(W_hh[l], np.float32).T) for l in range(L)]
    wa_t = [lhsT_layout(np.asarray(Wa[i], np.float32)) for i in range(L)]
    va_s = np.ascontiguousarray(
        np.asarray(va, np.float32).T.reshape(ACH, 128, L).transpose(1, 0, 2).astype(bf))

    bsum = np.asarray(b_ih, np.float32) + np.asarray(b_hh, np.float32)
    brz = np.ascontiguousarray(
        bsum[:, :1024].reshape(L, 8, 128).transpose(0, 2, 1))
    bin_ = np.ascontiguousarray(
        np.asarray(b_ih, np.float32)[:, 1024:].reshape(L, HT, 128).transpose(0, 2, 1))
    bhn = np.ascontiguousarray(
        np.asarray(b_hh, np.float32)[:, 1024:].reshape(L, HT, 128).transpose(0, 2, 1))

    ba_s = np.ascontiguousarray(
        np.asarray(ba, np.float32).T.reshape(ACH, 128, L).transpose(1, 0, 2))
    return emb_bf, wih_t, whh_t, wa_t, va_s, ba_s, brz, bin_, bhn


def kernel(tokens, emb, W_ih, W_hh, b_ih, b_hh, Wa, ba, va):
    nc = _get_nc()
    emb_bf, wih_t, whh_t, wa_t, va_s, ba_s, brz, bin_, bhn = _prep_inputs(
        tokens, emb, W_ih, W_hh, b_ih, b_hh, Wa, ba, va)

    tok = np.asarray(tokens).astype(np.int32)  # [T, B]
    wih_arr = np.stack(wih_t[1:])
    whh_arr = np.stack(whh_t)
    wa_arr = np.stack(wa_t)

    in_maps = []
    for c in range(NCORES):
        tok_c = np.ascontiguousarray(
            tok[:, c * BC:(c + 1) * BC]).reshape(TOK // 128, 128)
        in_maps.append({
            "tokens32": tok_c,
            "embbf": emb_bf,
            "wih0": wih_t[0],
            "wih": wih_arr,
            "whh": whh_arr,
            "wa": wa_arr,
            "vastk": va_s,
            "bastk": ba_s,
            "brz": brz,
            "bin": bin_,
            "bhn": bhn,
        })

    trace = bool(int(os.environ.get("KERNEL_TRACE", "0")))
    res = run_bass_kernel_spmd(nc, in_maps, core_ids=list(range(NCORES)),
                               trace=trace)
    if trace:
        _NC_CACHE["last_exec_time_ns"] = res.exec_time_ns
        _NC_CACHE["last_results"] = res

    outs = []
    for c in range(NCORES):
        o = res.results[c]["out"].reshape(T, BC, H)
        outs.append(o)
    return np.concatenate(outs, axis=1)
